# revision 3
# baseline (speedup 1.0000x reference)
import sys
import numpy as np

sys.path.insert(0, "/opt/trn_rl_repo")

from concourse import bacc, bass, tile
from concourse.bass_utils import run_bass_kernel_spmd

mybir = bass.mybir
import ml_dtypes

BF16 = ml_dtypes.bfloat16

B, T, F2, F1 = 64, 300, 2944, 1472
RED, H, K, LN = 256, 32, 15, 7
CA_RED = 320
N_CROP = T - 2 * (K - 1)
PYR = [2 ** j for j in range(1, LN)]

NCORES = 8
V = B // NCORES          # videos per core
R = V * T                # rows per core (2400)
NT = 23                  # xn k-tiles: exact var|mean, 23*128 == 2944
NT1 = 12                 # feat k-tiles (tile 11 only has 64 live rows)
NC = 480                 # column chunk (free dim per matmul)
NCHUNK = R // NC
G3 = 3 * H               # 96

# Feature order on device is permuted so that var/mean fuse pairs are
# partition-aligned: sigma = [var 0..1407 | mean 0..1407 | var 1408..1471 |
# mean 1408..1471]. Pair j pairs tile j with tile 11+j; tile 22 holds the
# last 64 features' var half (partitions 0..63) and mean half (64..127),
# which MM34 consumes as two base-64 k-tiles (the PSUM accumulate sums them).
SIG = np.concatenate([np.arange(0, 1408), 1472 + np.arange(0, 1408),
                      np.arange(1408, 1472), np.arange(2880, 2944)])

# MM2 m-tile emission order: tile 22 first (its prodL feeds MM34 via an
# SBUF->SBUF DMA with ~2us latency), then fuse pairs (j, 11+j).
M2_ORDER = [22] + [v for _j in range(11) for v in (_j, 11 + _j)]
assert sorted(M2_ORDER) == list(range(NT))

_bf = mybir.dt.bfloat16
_f32 = mybir.dt.float32


def _pool_matrix(N, sizes):
    mats = []
    for m in sizes:
        P = np.zeros((m, N), np.float32)
        for i in range(m):
            s = (i * N) // m
            e = -((-(i + 1) * N) // m)
            P[i, s:e] = 1.0 / (e - s)
        mats.append(P)
    return np.concatenate(mats, 0)


def _rne_bf16(x):
    """fp32 ndarray -> uint16 bf16 bits, round-to-nearest-even."""
    u = np.ascontiguousarray(x, np.float32).view(np.uint32)
    return ((u + 0x7FFF + ((u >> 16) & 1)) >> 16).astype(np.uint16)


def _build_bass():
    nc = bacc.Bacc(None, target_bir_lowering=False)

    xn_d = nc.dram_tensor("xn", [128, NT, R], _bf, kind="ExternalInput")
    w1_d = nc.dram_tensor("w1", [128, NT, CA_RED], _bf, kind="ExternalInput")
    b1_d = nc.dram_tensor("b1", [128, 3], _f32, kind="ExternalInput")
    w2_d = nc.dram_tensor("w2", [128, 3, NT * 128], _bf, kind="ExternalInput")
    b2_d = nc.dram_tensor("b2", [128, NT], _f32, kind="ExternalInput")
    w34_d = nc.dram_tensor("w34", [128, NT1, G3], _bf, kind="ExternalInput")
    b34_d = nc.dram_tensor("b34", [G3, 1], _f32, kind="ExternalInput")
    out_d = nc.dram_tensor("xg", [G3, R], _f32, kind="ExternalOutput")

    AF = mybir.ActivationFunctionType
    M1 = [(0, 128), (128, 128), (256, 64)]   # MM1 m-tiles over CA_RED=320
    K2 = [(0, 128), (1, 128), (2, 64)]       # MM2 k-tiles over 320

    with tile.TileContext(nc) as tc:
        with (
            tc.tile_pool(name="wp", bufs=1) as wp,
            tc.tile_pool(name="xp", bufs=2) as xp,
            tc.tile_pool(name="hp", bufs=2) as hp,
            tc.tile_pool(name="gp", bufs=2) as gp,
            tc.tile_pool(name="rp", bufs=2) as rp,
            tc.tile_pool(name="fp", bufs=2) as fp,
            tc.tile_pool(name="tp", bufs=3) as tp,
            tc.tile_pool(name="op", bufs=2) as op,
            tc.tile_pool(name="pa", bufs=1, space="PSUM") as pa,
            tc.tile_pool(name="pb", bufs=4, space="PSUM") as pb,
            tc.tile_pool(name="pc", bufs=1, space="PSUM") as pc,
        ):
            # DMA issue order tracks first use: w1 + xn0 (MM1 of chunk 0,
            # split so the k-outer chunk-0 MM1 starts early), xn1 (MM1 of
            # chunk 1), then w2 (MM2 of chunk 0).
            w1 = wp.tile([128, NT, CA_RED], _bf)
            xn0 = xp.tile([128, NT, NC], _bf, name="xn")
            nc.sync.dma_start(w1[:, :6, :], w1_d[:, :6, :])
            nc.sync.dma_start(xn0[:, :3, :], xn_d[:, :3, 0:NC])
            nc.sync.dma_start(w1[:, 6:14, :], w1_d[:, 6:14, :])
            nc.sync.dma_start(xn0[:, 3:8, :], xn_d[:, 3:8, 0:NC])
            nc.sync.dma_start(w1[:, 14:, :], w1_d[:, 14:, :])
            for q0, q1 in [(8, 13), (13, 18), (18, NT)]:
                nc.sync.dma_start(xn0[:, q0:q1, :], xn_d[:, q0:q1, 0:NC])
            b1 = wp.tile([128, 3], _f32)
            nc.sync.dma_start(b1[:], b1_d[:])
            xn1 = xp.tile([128, NT, NC], _bf, name="xn")
            for q0, q1 in [(0, 5), (5, 10), (10, 14), (14, 18), (18, NT)]:
                nc.sync.dma_start(xn1[:, q0:q1, :], xn_d[:, q0:q1, NC:2 * NC])
            w2 = wp.tile([128, 3, NT * 128], _bf)
            nc.sync.dma_start(w2[:], w2_d[:])
            b2 = wp.tile([128, NT], _f32)
            nc.sync.dma_start(b2[:], b2_d[:])
            w34 = wp.tile([128, NT1, G3], _bf)
            nc.sync.dma_start(w34[:], w34_d[:])
            b34 = wp.tile([G3, 1], _f32)
            nc.sync.dma_start(b34[:], b34_d[:])

            def mm1(xn, k_outer):
                # MM1: h1[320, NC] = relu(W1^T-chain), K = NT*128.
                # chunk 0 runs k-outer (3 live accumulators) so compute
                # overlaps the startup DMAs; later chunks run m-outer.
                h1 = hp.tile([128, 3, NC], _bf, name="h1")
                pss = [pa.tile([128, NC], _f32, name=f"ps1_{i}") for i in range(3)]
                order = (
                    [(kt, mi) for kt in range(NT) for mi in range(3)]
                    if k_outer else
                    [(kt, mi) for mi in range(3) for kt in range(NT)]
                )
                for kt, mi in order:
                    mo, msz = M1[mi]
                    nc.tensor.matmul(
                        pss[mi][:msz, :], w1[:, kt, mo:mo + msz], xn[:, kt, :],
                        start=(kt == 0), stop=(kt == NT - 1))
                for mi, (mo, msz) in enumerate(M1):
                    nc.scalar.activation(h1[:msz, mi, :], pss[mi][:msz, :],
                                         AF.Relu, bias=b1[:msz, mi:mi + 1])
                return h1

            # software pipeline: MM1 of chunk c+1 runs between MM1(c)'s
            # activations and MM2(c), so the PE never waits on the scalar
            # engine at the MM1->MM2 handoff.
            xns = [xn0, xn1]
            h1s = [mm1(xn0, True)]
            for c in range(NCHUNK):
                c0 = c * NC
                if c + 1 < NCHUNK:
                    h1s.append(mm1(xns[c + 1], c == 0))
                xn, h1 = xns[c], h1s[c]

                # MM2: g[2944, NC] = sigmoid(...), K = 320; m-tiles emitted in
                # M2_ORDER; prod[mt] = g[mt] * xn[mt] on DVE right after each
                # sigmoid so fuse pairs complete early.
                # MM2 + fuse: feat[j] = g[j]*xn[j] + g[11+j]*xn[11+j], all
                # partition-aligned under sigma. Fuse ops are emitted as soon
                # as each pair's sigmoids exist so the DVE tracks MM2.
                g = gp.tile([128, NT, NC], _bf)
                prodL = rp.tile([128, NC], _bf, name="prodL")
                prodM = rp.tile([64, NC], _bf, name="prodM")
                feat = fp.tile([128, NT1, NC], _bf)
                for mt in M2_ORDER:
                    mo2 = mt * 128
                    ps = pb.tile([128, NC], _f32, name="ps2")
                    for ki, ksz in K2:
                        nc.tensor.matmul(
                            ps[:, :], w2[:ksz, ki, mo2:mo2 + 128], h1[:ksz, ki, :],
                            start=(ki == 0), stop=(ki == 2))
                    nc.scalar.activation(g[:, mt, :], ps[:, :],
                                         AF.Sigmoid, bias=b2[:, mt:mt + 1])
                    if mt < 11:
                        nc.vector.tensor_mul(feat[:, mt, :], g[:, mt, :], xn[:, mt, :])
                    elif mt < 22:
                        j = mt - 11
                        tmp = tp.tile([128, NC], _bf)
                        nc.vector.tensor_mul(tmp[:], g[:, mt, :], xn[:, mt, :])
                        nc.vector.tensor_add(feat[:, j, :], feat[:, j, :], tmp[:])
                    else:
                        nc.vector.tensor_mul(prodL[:], g[:, 22, :], xn[:, 22, :])
                        # partition-shifted compute ops crash the device;
                        # move the mean half to base 0 with an SBUF->SBUF DMA
                        nc.sync.dma_start(prodM[:, :], prodL[64:128, :])

                # MM34 (ANN fc0 + GRU input proj folded): xg[96, NC].
                # K = 11 full feat tiles + two base-64 k-tiles of prodL (the
                # var/mean halves of features 1408..1471; PSUM sums them).
                # Last chunk runs in two halves so the final activation + DMA
                # tail overlaps the second half's matmuls.
                halves = [(0, NC)] if c + 1 < NCHUNK else [(0, NC // 2), (NC // 2, NC)]
                ps = pc.tile([128, NC], _f32, name="ps34")
                xg = op.tile([G3, NC], _f32, name="xg")
                for h0, h1e in halves:
                    for kt in range(11):
                        nc.tensor.matmul(
                            ps[:G3, h0:h1e], w34[:, kt, :], feat[:, kt, h0:h1e],
                            start=(kt == 0), stop=False)
                    nc.tensor.matmul(ps[:G3, h0:h1e], w34[0:64, 11, :],
                                     prodL[0:64, h0:h1e], start=False, stop=False)
                    nc.tensor.matmul(ps[:G3, h0:h1e], w34[0:64, 11, :],
                                     prodM[:, h0:h1e], start=False, stop=True)
                    nc.scalar.activation(xg[:, h0:h1e], ps[:G3, h0:h1e],
                                         AF.Identity, bias=b34[:, 0:1])
                    nc.sync.dma_start(out_d[:, c0 + h0:c0 + h1e], xg[:, h0:h1e])
                if c + 2 < NCHUNK:
                    xn2 = xp.tile([128, NT, NC], _bf, name="xn")
                    nc.sync.dma_start(xn2[:], xn_d[:, :, c0 + 2 * NC:c0 + 3 * NC])
                    xns.append(xn2)

    nc.compile()
    return nc


_NC_CACHE = []
_LAST_RES = []


def make_dev_weights(ca_fc1_w, ca_fc1_b, ca_fc2_w, ca_fc2_b,
                     ann_w, ann_b, gru_w_ih, gru_b_ih):
    """Device-layout weight map (shared across cores)."""
    w1_dev = np.ascontiguousarray(
        ca_fc1_w[:, SIG].T.reshape(NT, 128, CA_RED).transpose(1, 0, 2))
    b1_dev = np.zeros((3, 128), np.float32)
    b1_dev.reshape(-1)[:CA_RED] = ca_fc1_b
    b1_dev = np.ascontiguousarray(b1_dev.T)                       # [128,3]

    w2_p = np.zeros((384, F2), np.float32)
    w2_p[:CA_RED] = ca_fc2_w[SIG, :].T
    w2_dev = np.ascontiguousarray(
        w2_p.reshape(3, 128, F2).transpose(1, 0, 2))              # [128,3,2944]
    b2_dev = np.ascontiguousarray(
        ca_fc2_b[SIG].reshape(NT, 128).T)                         # [128,NT]

    # ANN fc0 and the GRU input projection are both linear -> fold:
    # xg = feat @ (W_ih @ W_ann).T + (W_ih @ b_ann + b_ih)
    w34 = (gru_w_ih.astype(np.float64) @ ann_w.astype(np.float64)).astype(np.float32)
    b34 = (gru_w_ih.astype(np.float64) @ ann_b.astype(np.float64)
           + gru_b_ih.astype(np.float64)).astype(np.float32)
    w34_dev = np.zeros((128, NT1, G3), np.float32)
    w34_dev[:, :11, :] = w34[:, :1408].reshape(G3, 11, 128).transpose(2, 1, 0)
    w34_dev[0:64, 11, :] = w34[:, 1408:].T      # feats 1408..1471 (used twice)
    b34_dev = np.ascontiguousarray(b34.reshape(G3, 1))

    return {
        "w1": _rne_bf16(w1_dev).view(BF16),
        "b1": b1_dev,
        "w2": _rne_bf16(w2_dev).view(BF16),
        "b2": b2_dev,
        "w34": _rne_bf16(w34_dev).view(BF16),
        "b34": b34_dev,
    }


def make_xn_u16(flat, mean_var, std_var, mean_mean, std_mean):
    """[rows, F2] fp32 -> normalized bf16 bits [rows, F2] in sigma order."""
    rows = flat.shape[0]
    out = np.empty((rows, F2), np.uint16)
    inv_sv = (1.0 / std_var).astype(np.float32)
    inv_sm = (1.0 / std_mean).astype(np.float32)
    xv = _rne_bf16((flat[:, :F1] - mean_var) * inv_sv)
    xm = _rne_bf16((flat[:, F1:] - mean_mean) * inv_sm)
    out[:, 0:1408] = xv[:, :1408]
    out[:, 1408:2816] = xm[:, :1408]
    out[:, 2816:2880] = xv[:, 1408:]
    out[:, 2880:2944] = xm[:, 1408:]
    return out


def core_xn(xn_u16_rows):
    """[R, F2] u16 -> [128, NT, R] device layout."""
    return np.ascontiguousarray(
        xn_u16_rows.reshape(R, NT, 128).transpose(2, 1, 0))


def kernel(input, input_length, mean_var, std_var, mean_mean, std_mean,
           ca_fc1_w, ca_fc1_b, ca_fc2_w, ca_fc2_b, ann_w, ann_b,
           gru_w_ih, gru_w_hh, gru_b_ih, gru_b_hh, q_w, q_b,
           att_w1, att_b1, att_w2, att_b2, qreg2_w, qreg2_b):
    input = np.asarray(input, np.float32)

    xn_u = make_xn_u16(input.reshape(B * T, F2),
                       mean_var, std_var, mean_mean, std_mean)
    shared = make_dev_weights(ca_fc1_w, ca_fc1_b, ca_fc2_w, ca_fc2_b,
                              ann_w, ann_b, gru_w_ih, gru_b_ih)

    in_maps = []
    for c in range(NCORES):
        m = dict(shared)
        m["xn"] = core_xn(xn_u[c * R:(c + 1) * R]).view(BF16)
        in_maps.append(m)

    if not _NC_CACHE:
        _NC_CACHE.append(_build_bass())
    nc = _NC_CACHE[0]

    res = run_bass_kernel_spmd(nc, in_maps, core_ids=list(range(NCORES)))
    _LAST_RES.clear()
    _LAST_RES.append(res)
    xg = np.stack([r["xg"] for r in res.results])        # [NCORES,96,R]
    xg = np.transpose(xg, (0, 2, 1)).reshape(B, T, G3)   # [B,T,96]

    # --- host: GRU scan (tiny)
    w_hh_T = gru_w_hh.T.astype(np.float32)
    b_hh = gru_b_hh.astype(np.float32)
    h = np.zeros((B, H), np.float32)
    outs = np.empty((B, T, H), np.float32)
    for t in range(T):
        hg = h @ w_hh_T + b_hh
        xt = xg[:, t]
        r = 1.0 / (1.0 + np.exp(-(xt[:, :H] + hg[:, :H])))
        z = 1.0 / (1.0 + np.exp(-(xt[:, H:2 * H] + hg[:, H:2 * H])))
        n = np.tanh(xt[:, 2 * H:] + r * hg[:, 2 * H:])
        h = (1.0 - z) * n + z * h
        outs[:, t] = h

    # --- temporal attention convs
    sw = np.lib.stride_tricks.sliding_window_view
    win1 = sw(outs, K, axis=1)                           # [B,T-14,H,K]
    a = np.einsum("btck,ck->bt", win1, att_w1[0], optimize=True) + att_b1[0]
    a = np.maximum(a, 0.0)
    win2 = sw(a, K, axis=1)                              # [B,N_CROP,K]
    a = win2 @ att_w2[0, 0] + att_b2[0]
    att = np.tanh(a)                                     # [B,N_CROP]

    q_fet = att[..., None] * outs[:, K - 1:T - (K - 1), :]   # [B,N_CROP,H]

    P = _pool_matrix(N_CROP, PYR)                        # [126,N_CROP]
    row0 = q_fet.mean(1, keepdims=True)
    num = np.einsum("mn,bnh->bmh", P, q_fet, optimize=True)
    den = (att @ P.T)[..., None]
    q_pool = np.concatenate([row0, num / den], 1)        # [B,127,H]

    q_each = (q_pool @ q_w.T + q_b)[..., 0]              # [B,127]
    score = q_each @ qreg2_w.T + qreg2_b
    return score.astype(np.float32)


# revision 4
# speedup vs baseline: 1.0009x; 1.0009x over previous
import sys
import numpy as np

sys.path.insert(0, "/opt/trn_rl_repo")

from concourse import bacc, bass, tile
from concourse.bass_utils import run_bass_kernel_spmd

mybir = bass.mybir
import ml_dtypes

BF16 = ml_dtypes.bfloat16

B, T, F2, F1 = 64, 300, 2944, 1472
RED, H, K, LN = 256, 32, 15, 7
CA_RED = 320
N_CROP = T - 2 * (K - 1)
PYR = [2 ** j for j in range(1, LN)]

NCORES = 8
V = B // NCORES          # videos per core
R = V * T                # rows per core (2400)
NT = 23                  # xn k-tiles: exact var|mean, 23*128 == 2944
NT1 = 12                 # feat k-tiles (tile 11 only has 64 live rows)
NC = 480                 # column chunk (free dim per matmul)
NCHUNK = R // NC
G3 = 3 * H               # 96

# Feature order on device is permuted so that var/mean fuse pairs are
# partition-aligned: sigma = [var 0..1407 | mean 0..1407 | var 1408..1471 |
# mean 1408..1471]. Pair j pairs tile j with tile 11+j; tile 22 holds the
# last 64 features' var half (partitions 0..63) and mean half (64..127),
# which MM34 consumes as two base-64 k-tiles (the PSUM accumulate sums them).
SIG = np.concatenate([np.arange(0, 1408), 1472 + np.arange(0, 1408),
                      np.arange(1408, 1472), np.arange(2880, 2944)])

# MM2 m-tile emission order: tile 22 first (its prodL feeds MM34 via an
# SBUF->SBUF DMA with ~2us latency), then fuse pairs (j, 11+j).
M2_ORDER = [22] + [v for _j in range(11) for v in (_j, 11 + _j)]
assert sorted(M2_ORDER) == list(range(NT))

_bf = mybir.dt.bfloat16
_f32 = mybir.dt.float32


def _pool_matrix(N, sizes):
    mats = []
    for m in sizes:
        P = np.zeros((m, N), np.float32)
        for i in range(m):
            s = (i * N) // m
            e = -((-(i + 1) * N) // m)
            P[i, s:e] = 1.0 / (e - s)
        mats.append(P)
    return np.concatenate(mats, 0)


def _rne_bf16(x):
    """fp32 ndarray -> uint16 bf16 bits, round-to-nearest-even."""
    u = np.ascontiguousarray(x, np.float32).view(np.uint32)
    return ((u + 0x7FFF + ((u >> 16) & 1)) >> 16).astype(np.uint16)


def _build_bass():
    nc = bacc.Bacc(None, target_bir_lowering=False)

    xn_d = nc.dram_tensor("xn", [128, NT, R], _bf, kind="ExternalInput")
    w1_d = nc.dram_tensor("w1", [128, NT, CA_RED], _bf, kind="ExternalInput")
    b1_d = nc.dram_tensor("b1", [128, 3], _f32, kind="ExternalInput")
    w2_d = nc.dram_tensor("w2", [128, 3, NT * 128], _bf, kind="ExternalInput")
    b2_d = nc.dram_tensor("b2", [128, NT], _f32, kind="ExternalInput")
    w34_d = nc.dram_tensor("w34", [128, NT1, G3], _bf, kind="ExternalInput")
    b34_d = nc.dram_tensor("b34", [G3, 1], _f32, kind="ExternalInput")
    out_d = nc.dram_tensor("xg", [G3, R], _f32, kind="ExternalOutput")

    AF = mybir.ActivationFunctionType
    M1 = [(0, 128), (128, 128), (256, 64)]   # MM1 m-tiles over CA_RED=320
    K2 = [(0, 128), (1, 128), (2, 64)]       # MM2 k-tiles over 320

    with tile.TileContext(nc) as tc:
        with (
            tc.tile_pool(name="wp", bufs=1) as wp,
            tc.tile_pool(name="xp", bufs=2) as xp,
            tc.tile_pool(name="hp", bufs=2) as hp,
            tc.tile_pool(name="gp", bufs=2) as gp,
            tc.tile_pool(name="rp", bufs=2) as rp,
            tc.tile_pool(name="fp", bufs=2) as fp,
            tc.tile_pool(name="tp", bufs=3) as tp,
            tc.tile_pool(name="op", bufs=2) as op,
            tc.tile_pool(name="pa", bufs=1, space="PSUM") as pa,
            tc.tile_pool(name="pb", bufs=4, space="PSUM") as pb,
            tc.tile_pool(name="pc", bufs=1, space="PSUM") as pc,
        ):
            # DMA issue order tracks first use: w1 + xn0 (MM1 of chunk 0,
            # split so the k-outer chunk-0 MM1 starts early), xn1 (MM1 of
            # chunk 1), then w2 (MM2 of chunk 0).
            w1 = wp.tile([128, NT, CA_RED], _bf)
            xn0 = xp.tile([128, NT, NC], _bf, name="xn")
            nc.sync.dma_start(w1[:, :3, :], w1_d[:, :3, :])
            nc.sync.dma_start(xn0[:, :3, :], xn_d[:, :3, 0:NC])
            nc.sync.dma_start(w1[:, 3:9, :], w1_d[:, 3:9, :])
            nc.sync.dma_start(xn0[:, 3:8, :], xn_d[:, 3:8, 0:NC])
            nc.sync.dma_start(w1[:, 9:16, :], w1_d[:, 9:16, :])
            nc.sync.dma_start(w1[:, 16:, :], w1_d[:, 16:, :])
            for q0, q1 in [(8, 13), (13, 18), (18, NT)]:
                nc.sync.dma_start(xn0[:, q0:q1, :], xn_d[:, q0:q1, 0:NC])
            b1 = wp.tile([128, 3], _f32)
            nc.sync.dma_start(b1[:], b1_d[:])
            xn1 = xp.tile([128, NT, NC], _bf, name="xn")
            for q0, q1 in [(0, 5), (5, 10), (10, 14), (14, 18), (18, NT)]:
                nc.sync.dma_start(xn1[:, q0:q1, :], xn_d[:, q0:q1, NC:2 * NC])
            w2 = wp.tile([128, 3, NT * 128], _bf)
            nc.sync.dma_start(w2[:], w2_d[:])
            b2 = wp.tile([128, NT], _f32)
            nc.sync.dma_start(b2[:], b2_d[:])
            w34 = wp.tile([128, NT1, G3], _bf)
            nc.sync.dma_start(w34[:], w34_d[:])
            b34 = wp.tile([G3, 1], _f32)
            nc.sync.dma_start(b34[:], b34_d[:])

            def mm1(xn, k_outer):
                # MM1: h1[320, NC] = relu(W1^T-chain), K = NT*128.
                # chunk 0 runs k-outer (3 live accumulators) so compute
                # overlaps the startup DMAs; later chunks run m-outer.
                h1 = hp.tile([128, 3, NC], _bf, name="h1")
                pss = [pa.tile([128, NC], _f32, name=f"ps1_{i}") for i in range(3)]
                order = (
                    [(kt, mi) for kt in range(NT) for mi in range(3)]
                    if k_outer else
                    [(kt, mi) for mi in range(3) for kt in range(NT)]
                )
                for kt, mi in order:
                    mo, msz = M1[mi]
                    nc.tensor.matmul(
                        pss[mi][:msz, :], w1[:, kt, mo:mo + msz], xn[:, kt, :],
                        start=(kt == 0), stop=(kt == NT - 1))
                for mi, (mo, msz) in enumerate(M1):
                    nc.scalar.activation(h1[:msz, mi, :], pss[mi][:msz, :],
                                         AF.Relu, bias=b1[:msz, mi:mi + 1])
                return h1

            # software pipeline: MM1 of chunk c+1 runs between MM1(c)'s
            # activations and MM2(c), so the PE never waits on the scalar
            # engine at the MM1->MM2 handoff.
            xns = [xn0, xn1]
            h1s = [mm1(xn0, True)]
            for c in range(NCHUNK):
                c0 = c * NC
                if c + 1 < NCHUNK:
                    h1s.append(mm1(xns[c + 1], c == 0))
                xn, h1 = xns[c], h1s[c]

                # MM2: g[2944, NC] = sigmoid(...), K = 320; m-tiles emitted in
                # M2_ORDER; prod[mt] = g[mt] * xn[mt] on DVE right after each
                # sigmoid so fuse pairs complete early.
                # MM2 + fuse: feat[j] = g[j]*xn[j] + g[11+j]*xn[11+j], all
                # partition-aligned under sigma. Fuse ops are emitted as soon
                # as each pair's sigmoids exist so the DVE tracks MM2.
                g = gp.tile([128, NT, NC], _bf)
                prodL = rp.tile([128, NC], _bf, name="prodL")
                prodM = rp.tile([64, NC], _bf, name="prodM")
                feat = fp.tile([128, NT1, NC], _bf)
                for mt in M2_ORDER:
                    mo2 = mt * 128
                    ps = pb.tile([128, NC], _f32, name="ps2")
                    for ki, ksz in K2:
                        nc.tensor.matmul(
                            ps[:, :], w2[:ksz, ki, mo2:mo2 + 128], h1[:ksz, ki, :],
                            start=(ki == 0), stop=(ki == 2))
                    nc.scalar.activation(g[:, mt, :], ps[:, :],
                                         AF.Sigmoid, bias=b2[:, mt:mt + 1])
                    if mt < 11:
                        nc.vector.tensor_mul(feat[:, mt, :], g[:, mt, :], xn[:, mt, :])
                    elif mt < 22:
                        j = mt - 11
                        tmp = tp.tile([128, NC], _bf)
                        nc.vector.tensor_mul(tmp[:], g[:, mt, :], xn[:, mt, :])
                        nc.vector.tensor_add(feat[:, j, :], feat[:, j, :], tmp[:])
                    else:
                        nc.vector.tensor_mul(prodL[:], g[:, 22, :], xn[:, 22, :])
                        # partition-shifted compute ops crash the device;
                        # move the mean half to base 0 with an SBUF->SBUF DMA
                        nc.sync.dma_start(prodM[:, :], prodL[64:128, :])

                # MM34 (ANN fc0 + GRU input proj folded): xg[96, NC].
                # K = 11 full feat tiles + two base-64 k-tiles of prodL (the
                # var/mean halves of features 1408..1471; PSUM sums them).
                # Last chunk runs in two halves so the final activation + DMA
                # tail overlaps the second half's matmuls.
                halves = [(0, NC)] if c + 1 < NCHUNK else [(0, NC // 2), (NC // 2, NC)]
                ps = pc.tile([128, NC], _f32, name="ps34")
                xg = op.tile([G3, NC], _f32, name="xg")
                for h0, h1e in halves:
                    for kt in range(11):
                        nc.tensor.matmul(
                            ps[:G3, h0:h1e], w34[:, kt, :], feat[:, kt, h0:h1e],
                            start=(kt == 0), stop=False)
                    nc.tensor.matmul(ps[:G3, h0:h1e], w34[0:64, 11, :],
                                     prodL[0:64, h0:h1e], start=False, stop=False)
                    nc.tensor.matmul(ps[:G3, h0:h1e], w34[0:64, 11, :],
                                     prodM[:, h0:h1e], start=False, stop=True)
                    nc.scalar.activation(xg[:, h0:h1e], ps[:G3, h0:h1e],
                                         AF.Identity, bias=b34[:, 0:1])
                    nc.sync.dma_start(out_d[:, c0 + h0:c0 + h1e], xg[:, h0:h1e])
                if c + 2 < NCHUNK:
                    xn2 = xp.tile([128, NT, NC], _bf, name="xn")
                    nc.sync.dma_start(xn2[:], xn_d[:, :, c0 + 2 * NC:c0 + 3 * NC])
                    xns.append(xn2)

    nc.compile()
    return nc


_NC_CACHE = []
_LAST_RES = []


def make_dev_weights(ca_fc1_w, ca_fc1_b, ca_fc2_w, ca_fc2_b,
                     ann_w, ann_b, gru_w_ih, gru_b_ih):
    """Device-layout weight map (shared across cores)."""
    w1_dev = np.ascontiguousarray(
        ca_fc1_w[:, SIG].T.reshape(NT, 128, CA_RED).transpose(1, 0, 2))
    b1_dev = np.zeros((3, 128), np.float32)
    b1_dev.reshape(-1)[:CA_RED] = ca_fc1_b
    b1_dev = np.ascontiguousarray(b1_dev.T)                       # [128,3]

    w2_p = np.zeros((384, F2), np.float32)
    w2_p[:CA_RED] = ca_fc2_w[SIG, :].T
    w2_dev = np.ascontiguousarray(
        w2_p.reshape(3, 128, F2).transpose(1, 0, 2))              # [128,3,2944]
    b2_dev = np.ascontiguousarray(
        ca_fc2_b[SIG].reshape(NT, 128).T)                         # [128,NT]

    # ANN fc0 and the GRU input projection are both linear -> fold:
    # xg = feat @ (W_ih @ W_ann).T + (W_ih @ b_ann + b_ih)
    w34 = (gru_w_ih.astype(np.float64) @ ann_w.astype(np.float64)).astype(np.float32)
    b34 = (gru_w_ih.astype(np.float64) @ ann_b.astype(np.float64)
           + gru_b_ih.astype(np.float64)).astype(np.float32)
    w34_dev = np.zeros((128, NT1, G3), np.float32)
    w34_dev[:, :11, :] = w34[:, :1408].reshape(G3, 11, 128).transpose(2, 1, 0)
    w34_dev[0:64, 11, :] = w34[:, 1408:].T      # feats 1408..1471 (used twice)
    b34_dev = np.ascontiguousarray(b34.reshape(G3, 1))

    return {
        "w1": _rne_bf16(w1_dev).view(BF16),
        "b1": b1_dev,
        "w2": _rne_bf16(w2_dev).view(BF16),
        "b2": b2_dev,
        "w34": _rne_bf16(w34_dev).view(BF16),
        "b34": b34_dev,
    }


def make_xn_u16(flat, mean_var, std_var, mean_mean, std_mean):
    """[rows, F2] fp32 -> normalized bf16 bits [rows, F2] in sigma order."""
    rows = flat.shape[0]
    out = np.empty((rows, F2), np.uint16)
    inv_sv = (1.0 / std_var).astype(np.float32)
    inv_sm = (1.0 / std_mean).astype(np.float32)
    xv = _rne_bf16((flat[:, :F1] - mean_var) * inv_sv)
    xm = _rne_bf16((flat[:, F1:] - mean_mean) * inv_sm)
    out[:, 0:1408] = xv[:, :1408]
    out[:, 1408:2816] = xm[:, :1408]
    out[:, 2816:2880] = xv[:, 1408:]
    out[:, 2880:2944] = xm[:, 1408:]
    return out


def core_xn(xn_u16_rows):
    """[R, F2] u16 -> [128, NT, R] device layout."""
    return np.ascontiguousarray(
        xn_u16_rows.reshape(R, NT, 128).transpose(2, 1, 0))


def kernel(input, input_length, mean_var, std_var, mean_mean, std_mean,
           ca_fc1_w, ca_fc1_b, ca_fc2_w, ca_fc2_b, ann_w, ann_b,
           gru_w_ih, gru_w_hh, gru_b_ih, gru_b_hh, q_w, q_b,
           att_w1, att_b1, att_w2, att_b2, qreg2_w, qreg2_b):
    input = np.asarray(input, np.float32)

    xn_u = make_xn_u16(input.reshape(B * T, F2),
                       mean_var, std_var, mean_mean, std_mean)
    shared = make_dev_weights(ca_fc1_w, ca_fc1_b, ca_fc2_w, ca_fc2_b,
                              ann_w, ann_b, gru_w_ih, gru_b_ih)

    in_maps = []
    for c in range(NCORES):
        m = dict(shared)
        m["xn"] = core_xn(xn_u[c * R:(c + 1) * R]).view(BF16)
        in_maps.append(m)

    if not _NC_CACHE:
        _NC_CACHE.append(_build_bass())
    nc = _NC_CACHE[0]

    res = run_bass_kernel_spmd(nc, in_maps, core_ids=list(range(NCORES)))
    _LAST_RES.clear()
    _LAST_RES.append(res)
    xg = np.stack([r["xg"] for r in res.results])        # [NCORES,96,R]
    xg = np.transpose(xg, (0, 2, 1)).reshape(B, T, G3)   # [B,T,96]

    # --- host: GRU scan (tiny)
    w_hh_T = gru_w_hh.T.astype(np.float32)
    b_hh = gru_b_hh.astype(np.float32)
    h = np.zeros((B, H), np.float32)
    outs = np.empty((B, T, H), np.float32)
    for t in range(T):
        hg = h @ w_hh_T + b_hh
        xt = xg[:, t]
        r = 1.0 / (1.0 + np.exp(-(xt[:, :H] + hg[:, :H])))
        z = 1.0 / (1.0 + np.exp(-(xt[:, H:2 * H] + hg[:, H:2 * H])))
        n = np.tanh(xt[:, 2 * H:] + r * hg[:, 2 * H:])
        h = (1.0 - z) * n + z * h
        outs[:, t] = h

    # --- temporal attention convs
    sw = np.lib.stride_tricks.sliding_window_view
    win1 = sw(outs, K, axis=1)                           # [B,T-14,H,K]
    a = np.einsum("btck,ck->bt", win1, att_w1[0], optimize=True) + att_b1[0]
    a = np.maximum(a, 0.0)
    win2 = sw(a, K, axis=1)                              # [B,N_CROP,K]
    a = win2 @ att_w2[0, 0] + att_b2[0]
    att = np.tanh(a)                                     # [B,N_CROP]

    q_fet = att[..., None] * outs[:, K - 1:T - (K - 1), :]   # [B,N_CROP,H]

    P = _pool_matrix(N_CROP, PYR)                        # [126,N_CROP]
    row0 = q_fet.mean(1, keepdims=True)
    num = np.einsum("mn,bnh->bmh", P, q_fet, optimize=True)
    den = (att @ P.T)[..., None]
    q_pool = np.concatenate([row0, num / den], 1)        # [B,127,H]

    q_each = (q_pool @ q_w.T + q_b)[..., 0]              # [B,127]
    score = q_each @ qreg2_w.T + qreg2_b
    return score.astype(np.float32)


# revision 5
# speedup vs baseline: 1.0083x; 1.0074x over previous
import sys
import numpy as np

sys.path.insert(0, "/opt/trn_rl_repo")

from concourse import bacc, bass, tile
from concourse.bass_utils import run_bass_kernel_spmd

mybir = bass.mybir
import ml_dtypes

BF16 = ml_dtypes.bfloat16

B, T, F2, F1 = 64, 300, 2944, 1472
RED, H, K, LN = 256, 32, 15, 7
CA_RED = 320
N_CROP = T - 2 * (K - 1)
PYR = [2 ** j for j in range(1, LN)]

NCORES = 8
V = B // NCORES          # videos per core
R = V * T                # rows per core (2400)
NT = 23                  # xn k-tiles: exact var|mean, 23*128 == 2944
NT1 = 12                 # feat k-tiles (tile 11 only has 64 live rows)
NC = 480                 # column chunk (free dim per matmul)
NCHUNK = R // NC
G3 = 3 * H               # 96

# Feature order on device is permuted so that var/mean fuse pairs are
# partition-aligned: sigma = [var 0..1407 | mean 0..1407 | var 1408..1471 |
# mean 1408..1471]. Pair j pairs tile j with tile 11+j; tile 22 holds the
# last 64 features' var half (partitions 0..63) and mean half (64..127),
# which MM34 consumes as two base-64 k-tiles (the PSUM accumulate sums them).
SIG = np.concatenate([np.arange(0, 1408), 1472 + np.arange(0, 1408),
                      np.arange(1408, 1472), np.arange(2880, 2944)])

# MM2 m-tile emission order: tile 22 first (its prodL feeds MM34 via an
# SBUF->SBUF DMA with ~2us latency), then fuse pairs (j, 11+j).
M2_ORDER = [22] + [v for _j in range(11) for v in (_j, 11 + _j)]
assert sorted(M2_ORDER) == list(range(NT))

_bf = mybir.dt.bfloat16
_f32 = mybir.dt.float32


def _pool_matrix(N, sizes):
    mats = []
    for m in sizes:
        P = np.zeros((m, N), np.float32)
        for i in range(m):
            s = (i * N) // m
            e = -((-(i + 1) * N) // m)
            P[i, s:e] = 1.0 / (e - s)
        mats.append(P)
    return np.concatenate(mats, 0)


def _rne_bf16(x):
    """fp32 ndarray -> uint16 bf16 bits, round-to-nearest-even."""
    u = np.ascontiguousarray(x, np.float32).view(np.uint32)
    return ((u + 0x7FFF + ((u >> 16) & 1)) >> 16).astype(np.uint16)


def _build_bass():
    nc = bacc.Bacc(None, target_bir_lowering=False)

    xn_d = nc.dram_tensor("xn", [128, NT, R], _bf, kind="ExternalInput")
    w1_d = nc.dram_tensor("w1", [128, NT, CA_RED], _bf, kind="ExternalInput")
    b1_d = nc.dram_tensor("b1", [128, 3], _f32, kind="ExternalInput")
    w2_d = nc.dram_tensor("w2", [128, 3, NT * 128], _bf, kind="ExternalInput")
    b2_d = nc.dram_tensor("b2", [128, NT], _f32, kind="ExternalInput")
    w34_d = nc.dram_tensor("w34", [128, NT1, G3], _bf, kind="ExternalInput")
    b34_d = nc.dram_tensor("b34", [G3, 1], _f32, kind="ExternalInput")
    out_d = nc.dram_tensor("xg", [G3, R], _f32, kind="ExternalOutput")

    AF = mybir.ActivationFunctionType
    M1 = [(0, 128), (128, 128), (256, 64)]   # MM1 m-tiles over CA_RED=320
    K2 = [(0, 128), (1, 128), (2, 64)]       # MM2 k-tiles over 320

    with tile.TileContext(nc) as tc:
        with (
            tc.tile_pool(name="wp", bufs=1) as wp,
            tc.tile_pool(name="xp", bufs=2) as xp,
            tc.tile_pool(name="hp", bufs=2) as hp,
            tc.tile_pool(name="gp", bufs=2) as gp,
            tc.tile_pool(name="rp", bufs=2) as rp,
            tc.tile_pool(name="fp", bufs=2) as fp,
            tc.tile_pool(name="tp", bufs=3) as tp,
            tc.tile_pool(name="op", bufs=2) as op,
            tc.tile_pool(name="pa", bufs=1, space="PSUM") as pa,
            tc.tile_pool(name="pb", bufs=4, space="PSUM") as pb,
            tc.tile_pool(name="pc", bufs=1, space="PSUM") as pc,
        ):
            # DMA issue order tracks first use: w1 + xn0 (MM1 of chunk 0,
            # split so the k-outer chunk-0 MM1 starts early), xn1 (MM1 of
            # chunk 1), then w2 (MM2 of chunk 0).
            w1 = wp.tile([128, NT, CA_RED], _bf)
            xn0 = xp.tile([128, NT, NC], _bf, name="xn")
            nc.sync.dma_start(w1[:, :3, :], w1_d[:, :3, :])
            nc.sync.dma_start(xn0[:, :3, :], xn_d[:, :3, 0:NC])
            nc.sync.dma_start(w1[:, 3:9, :], w1_d[:, 3:9, :])
            nc.sync.dma_start(xn0[:, 3:8, :], xn_d[:, 3:8, 0:NC])
            nc.sync.dma_start(w1[:, 9:16, :], w1_d[:, 9:16, :])
            nc.sync.dma_start(w1[:, 16:, :], w1_d[:, 16:, :])
            for q0, q1 in [(8, 13), (13, 18), (18, NT)]:
                nc.sync.dma_start(xn0[:, q0:q1, :], xn_d[:, q0:q1, 0:NC])
            b1 = wp.tile([128, 3], _f32)
            nc.sync.dma_start(b1[:], b1_d[:])
            xn1 = xp.tile([128, NT, NC], _bf, name="xn")
            for q0, q1 in [(0, 5), (5, 10), (10, 14), (14, 18), (18, NT)]:
                nc.sync.dma_start(xn1[:, q0:q1, :], xn_d[:, q0:q1, NC:2 * NC])
            w2 = wp.tile([128, 3, NT * 128], _bf)
            nc.sync.dma_start(w2[:], w2_d[:])
            b2 = wp.tile([128, NT], _f32)
            nc.sync.dma_start(b2[:], b2_d[:])
            w34 = wp.tile([128, NT1, G3], _bf)
            nc.sync.dma_start(w34[:], w34_d[:])
            b34 = wp.tile([G3, 1], _f32)
            nc.sync.dma_start(b34[:], b34_d[:])

            def mm1(xn, k_outer):
                # MM1: h1[320, NC] = relu(W1^T-chain), K = NT*128.
                # chunk 0 runs k-outer (3 live accumulators) so compute
                # overlaps the startup DMAs; later chunks run m-outer.
                h1 = hp.tile([128, 3, NC], _bf, name="h1")
                pss = [pa.tile([128, NC], _f32, name=f"ps1_{i}") for i in range(3)]
                order = (
                    [(kt, mi) for kt in range(NT) for mi in range(3)]
                    if k_outer else
                    [(kt, mi) for mi in range(3) for kt in range(NT)]
                )
                for kt, mi in order:
                    mo, msz = M1[mi]
                    nc.tensor.matmul(
                        pss[mi][:msz, :], w1[:, kt, mo:mo + msz], xn[:, kt, :],
                        start=(kt == 0), stop=(kt == NT - 1))
                for mi, (mo, msz) in enumerate(M1):
                    nc.scalar.activation(h1[:msz, mi, :], pss[mi][:msz, :],
                                         AF.Relu, bias=b1[:msz, mi:mi + 1])
                return h1

            # software pipeline: MM1 of chunk c+1 runs between MM1(c)'s
            # activations and MM2(c), so the PE never waits on the scalar
            # engine at the MM1->MM2 handoff.
            xns = [xn0, xn1]
            h1s = [mm1(xn0, True)]
            for c in range(NCHUNK):
                c0 = c * NC
                if c + 1 < NCHUNK:
                    h1s.append(mm1(xns[c + 1], c == 0))
                xn, h1 = xns[c], h1s[c]

                # MM2: g[2944, NC] = sigmoid(...), K = 320; m-tiles emitted in
                # M2_ORDER; prod[mt] = g[mt] * xn[mt] on DVE right after each
                # sigmoid so fuse pairs complete early.
                # MM2 + fuse: feat[j] = g[j]*xn[j] + g[11+j]*xn[11+j], all
                # partition-aligned under sigma. Fuse ops are emitted as soon
                # as each pair's sigmoids exist so the DVE tracks MM2.
                g = gp.tile([128, NT, NC], _bf)
                prodL = rp.tile([128, NC], _bf, name="prodL")
                prodM = rp.tile([64, NC], _bf, name="prodM")
                feat = fp.tile([128, NT1, NC], _bf)
                for mt in M2_ORDER:
                    mo2 = mt * 128
                    ps = pb.tile([128, NC], _f32, name="ps2")
                    for ki, ksz in K2:
                        nc.tensor.matmul(
                            ps[:, :], w2[:ksz, ki, mo2:mo2 + 128], h1[:ksz, ki, :],
                            start=(ki == 0), stop=(ki == 2))
                    nc.scalar.activation(g[:, mt, :], ps[:, :],
                                         AF.Sigmoid, bias=b2[:, mt:mt + 1])
                    if mt < 11:
                        nc.vector.tensor_mul(feat[:, mt, :], g[:, mt, :], xn[:, mt, :])
                    elif mt < 22:
                        j = mt - 11
                        tmp = tp.tile([128, NC], _bf)
                        nc.vector.tensor_mul(tmp[:], g[:, mt, :], xn[:, mt, :])
                        nc.vector.tensor_add(feat[:, j, :], feat[:, j, :], tmp[:])
                    else:
                        nc.vector.tensor_mul(prodL[:], g[:, 22, :], xn[:, 22, :])
                        # partition-shifted compute ops crash the device;
                        # move the mean half to base 0 with an SBUF->SBUF DMA
                        nc.sync.dma_start(prodM[:, :], prodL[64:128, :])

                # MM34 (ANN fc0 + GRU input proj folded): xg[96, NC].
                # K = 11 full feat tiles + two base-64 k-tiles of prodL (the
                # var/mean halves of features 1408..1471; PSUM sums them).
                # Last chunk runs in two halves so the final activation + DMA
                # tail overlaps the second half's matmuls.
                # last chunk: split 360/120 and give each piece its OWN
                # PSUM tile (pa's ps1_1 is idle then) — a shared tile's
                # coarse dependency made piece 2 wait on piece 1's act.
                halves = [(0, NC, None)] if c + 1 < NCHUNK else                     [(0, 360, None), (360, NC, "ps1_1")]
                xg = op.tile([G3, NC], _f32, name="xg")
                for h0, h1e, alt in halves:
                    if alt is None:
                        ps = pc.tile([128, NC], _f32, name="ps34")
                    else:
                        ps = pa.tile([128, NC], _f32, name=alt)
                    for kt in range(11):
                        nc.tensor.matmul(
                            ps[:G3, h0:h1e], w34[:, kt, :], feat[:, kt, h0:h1e],
                            start=(kt == 0), stop=False)
                    nc.tensor.matmul(ps[:G3, h0:h1e], w34[0:64, 11, :],
                                     prodL[0:64, h0:h1e], start=False, stop=False)
                    nc.tensor.matmul(ps[:G3, h0:h1e], w34[0:64, 11, :],
                                     prodM[:, h0:h1e], start=False, stop=True)
                    nc.scalar.activation(xg[:, h0:h1e], ps[:G3, h0:h1e],
                                         AF.Identity, bias=b34[:, 0:1])
                    nc.sync.dma_start(out_d[:, c0 + h0:c0 + h1e], xg[:, h0:h1e])
                if c + 2 < NCHUNK:
                    xn2 = xp.tile([128, NT, NC], _bf, name="xn")
                    nc.sync.dma_start(xn2[:], xn_d[:, :, c0 + 2 * NC:c0 + 3 * NC])
                    xns.append(xn2)

    nc.compile()
    return nc


_NC_CACHE = []
_LAST_RES = []


def make_dev_weights(ca_fc1_w, ca_fc1_b, ca_fc2_w, ca_fc2_b,
                     ann_w, ann_b, gru_w_ih, gru_b_ih):
    """Device-layout weight map (shared across cores)."""
    w1_dev = np.ascontiguousarray(
        ca_fc1_w[:, SIG].T.reshape(NT, 128, CA_RED).transpose(1, 0, 2))
    b1_dev = np.zeros((3, 128), np.float32)
    b1_dev.reshape(-1)[:CA_RED] = ca_fc1_b
    b1_dev = np.ascontiguousarray(b1_dev.T)                       # [128,3]

    w2_p = np.zeros((384, F2), np.float32)
    w2_p[:CA_RED] = ca_fc2_w[SIG, :].T
    w2_dev = np.ascontiguousarray(
        w2_p.reshape(3, 128, F2).transpose(1, 0, 2))              # [128,3,2944]
    b2_dev = np.ascontiguousarray(
        ca_fc2_b[SIG].reshape(NT, 128).T)                         # [128,NT]

    # ANN fc0 and the GRU input projection are both linear -> fold:
    # xg = feat @ (W_ih @ W_ann).T + (W_ih @ b_ann + b_ih)
    w34 = (gru_w_ih.astype(np.float64) @ ann_w.astype(np.float64)).astype(np.float32)
    b34 = (gru_w_ih.astype(np.float64) @ ann_b.astype(np.float64)
           + gru_b_ih.astype(np.float64)).astype(np.float32)
    w34_dev = np.zeros((128, NT1, G3), np.float32)
    w34_dev[:, :11, :] = w34[:, :1408].reshape(G3, 11, 128).transpose(2, 1, 0)
    w34_dev[0:64, 11, :] = w34[:, 1408:].T      # feats 1408..1471 (used twice)
    b34_dev = np.ascontiguousarray(b34.reshape(G3, 1))

    return {
        "w1": _rne_bf16(w1_dev).view(BF16),
        "b1": b1_dev,
        "w2": _rne_bf16(w2_dev).view(BF16),
        "b2": b2_dev,
        "w34": _rne_bf16(w34_dev).view(BF16),
        "b34": b34_dev,
    }


def make_xn_u16(flat, mean_var, std_var, mean_mean, std_mean):
    """[rows, F2] fp32 -> normalized bf16 bits [rows, F2] in sigma order."""
    rows = flat.shape[0]
    out = np.empty((rows, F2), np.uint16)
    inv_sv = (1.0 / std_var).astype(np.float32)
    inv_sm = (1.0 / std_mean).astype(np.float32)
    xv = _rne_bf16((flat[:, :F1] - mean_var) * inv_sv)
    xm = _rne_bf16((flat[:, F1:] - mean_mean) * inv_sm)
    out[:, 0:1408] = xv[:, :1408]
    out[:, 1408:2816] = xm[:, :1408]
    out[:, 2816:2880] = xv[:, 1408:]
    out[:, 2880:2944] = xm[:, 1408:]
    return out


def core_xn(xn_u16_rows):
    """[R, F2] u16 -> [128, NT, R] device layout."""
    return np.ascontiguousarray(
        xn_u16_rows.reshape(R, NT, 128).transpose(2, 1, 0))


def kernel(input, input_length, mean_var, std_var, mean_mean, std_mean,
           ca_fc1_w, ca_fc1_b, ca_fc2_w, ca_fc2_b, ann_w, ann_b,
           gru_w_ih, gru_w_hh, gru_b_ih, gru_b_hh, q_w, q_b,
           att_w1, att_b1, att_w2, att_b2, qreg2_w, qreg2_b):
    input = np.asarray(input, np.float32)

    xn_u = make_xn_u16(input.reshape(B * T, F2),
                       mean_var, std_var, mean_mean, std_mean)
    shared = make_dev_weights(ca_fc1_w, ca_fc1_b, ca_fc2_w, ca_fc2_b,
                              ann_w, ann_b, gru_w_ih, gru_b_ih)

    in_maps = []
    for c in range(NCORES):
        m = dict(shared)
        m["xn"] = core_xn(xn_u[c * R:(c + 1) * R]).view(BF16)
        in_maps.append(m)

    if not _NC_CACHE:
        _NC_CACHE.append(_build_bass())
    nc = _NC_CACHE[0]

    res = run_bass_kernel_spmd(nc, in_maps, core_ids=list(range(NCORES)))
    _LAST_RES.clear()
    _LAST_RES.append(res)
    xg = np.stack([r["xg"] for r in res.results])        # [NCORES,96,R]
    xg = np.transpose(xg, (0, 2, 1)).reshape(B, T, G3)   # [B,T,96]

    # --- host: GRU scan (tiny)
    w_hh_T = gru_w_hh.T.astype(np.float32)
    b_hh = gru_b_hh.astype(np.float32)
    h = np.zeros((B, H), np.float32)
    outs = np.empty((B, T, H), np.float32)
    for t in range(T):
        hg = h @ w_hh_T + b_hh
        xt = xg[:, t]
        r = 1.0 / (1.0 + np.exp(-(xt[:, :H] + hg[:, :H])))
        z = 1.0 / (1.0 + np.exp(-(xt[:, H:2 * H] + hg[:, H:2 * H])))
        n = np.tanh(xt[:, 2 * H:] + r * hg[:, 2 * H:])
        h = (1.0 - z) * n + z * h
        outs[:, t] = h

    # --- temporal attention convs
    sw = np.lib.stride_tricks.sliding_window_view
    win1 = sw(outs, K, axis=1)                           # [B,T-14,H,K]
    a = np.einsum("btck,ck->bt", win1, att_w1[0], optimize=True) + att_b1[0]
    a = np.maximum(a, 0.0)
    win2 = sw(a, K, axis=1)                              # [B,N_CROP,K]
    a = win2 @ att_w2[0, 0] + att_b2[0]
    att = np.tanh(a)                                     # [B,N_CROP]

    q_fet = att[..., None] * outs[:, K - 1:T - (K - 1), :]   # [B,N_CROP,H]

    P = _pool_matrix(N_CROP, PYR)                        # [126,N_CROP]
    row0 = q_fet.mean(1, keepdims=True)
    num = np.einsum("mn,bnh->bmh", P, q_fet, optimize=True)
    den = (att @ P.T)[..., None]
    q_pool = np.concatenate([row0, num / den], 1)        # [B,127,H]

    q_each = (q_pool @ q_w.T + q_b)[..., 0]              # [B,127]
    score = q_each @ qreg2_w.T + qreg2_b
    return score.astype(np.float32)


# revision 6
# speedup vs baseline: 1.0109x; 1.0026x over previous
import sys
import numpy as np

sys.path.insert(0, "/opt/trn_rl_repo")

from concourse import bacc, bass, tile
from concourse.bass_utils import run_bass_kernel_spmd

mybir = bass.mybir
import ml_dtypes

BF16 = ml_dtypes.bfloat16

B, T, F2, F1 = 64, 300, 2944, 1472
RED, H, K, LN = 256, 32, 15, 7
CA_RED = 320
N_CROP = T - 2 * (K - 1)
PYR = [2 ** j for j in range(1, LN)]

NCORES = 8
V = B // NCORES          # videos per core
R = V * T                # rows per core (2400)
NT = 23                  # xn k-tiles: exact var|mean, 23*128 == 2944
NT1 = 12                 # feat k-tiles (tile 11 only has 64 live rows)
NC = 480                 # column chunk (free dim per matmul)
NCHUNK = R // NC
G3 = 3 * H               # 96

# Feature order on device is permuted so that var/mean fuse pairs are
# partition-aligned: sigma = [var 0..1407 | mean 0..1407 | var 1408..1471 |
# mean 1408..1471]. Pair j pairs tile j with tile 11+j; tile 22 holds the
# last 64 features' var half (partitions 0..63) and mean half (64..127),
# which MM34 consumes as two base-64 k-tiles (the PSUM accumulate sums them).
SIG = np.concatenate([np.arange(0, 1408), 1472 + np.arange(0, 1408),
                      np.arange(1408, 1472), np.arange(2880, 2944)])

# MM2 m-tile emission order: tile 22 first (its prodL feeds MM34 via an
# SBUF->SBUF DMA with ~2us latency), then fuse pairs (j, 11+j).
M2_ORDER = [22] + [v for _j in range(11) for v in (_j, 11 + _j)]
assert sorted(M2_ORDER) == list(range(NT))

_bf = mybir.dt.bfloat16
_f32 = mybir.dt.float32


def _pool_matrix(N, sizes):
    mats = []
    for m in sizes:
        P = np.zeros((m, N), np.float32)
        for i in range(m):
            s = (i * N) // m
            e = -((-(i + 1) * N) // m)
            P[i, s:e] = 1.0 / (e - s)
        mats.append(P)
    return np.concatenate(mats, 0)


def _rne_bf16(x):
    """fp32 ndarray -> uint16 bf16 bits, round-to-nearest-even."""
    u = np.ascontiguousarray(x, np.float32).view(np.uint32)
    return ((u + 0x7FFF + ((u >> 16) & 1)) >> 16).astype(np.uint16)


def _build_bass():
    nc = bacc.Bacc(None, target_bir_lowering=False)

    xn_d = nc.dram_tensor("xn", [128, NT, R], _bf, kind="ExternalInput")
    w1_d = nc.dram_tensor("w1", [128, NT, CA_RED], _bf, kind="ExternalInput")
    b1_d = nc.dram_tensor("b1", [128, 3], _f32, kind="ExternalInput")
    w2_d = nc.dram_tensor("w2", [128, 3, NT * 128], _bf, kind="ExternalInput")
    b2_d = nc.dram_tensor("b2", [128, NT], _f32, kind="ExternalInput")
    w34_d = nc.dram_tensor("w34", [128, NT1, G3], _bf, kind="ExternalInput")
    b34_d = nc.dram_tensor("b34", [G3, 1], _f32, kind="ExternalInput")
    out_d = nc.dram_tensor("xg", [G3, R], _f32, kind="ExternalOutput")

    AF = mybir.ActivationFunctionType
    M1 = [(0, 128), (128, 128), (256, 64)]   # MM1 m-tiles over CA_RED=320
    K2 = [(0, 128), (1, 128), (2, 64)]       # MM2 k-tiles over 320

    with tile.TileContext(nc) as tc:
        with (
            tc.tile_pool(name="wp", bufs=1) as wp,
            tc.tile_pool(name="xp", bufs=2) as xp,
            tc.tile_pool(name="hp", bufs=2) as hp,
            tc.tile_pool(name="gp", bufs=2) as gp,
            tc.tile_pool(name="rp", bufs=2) as rp,
            tc.tile_pool(name="fp", bufs=2) as fp,
            tc.tile_pool(name="tp", bufs=3) as tp,
            tc.tile_pool(name="op", bufs=2) as op,
            tc.tile_pool(name="pa", bufs=1, space="PSUM") as pa,
            tc.tile_pool(name="pb", bufs=4, space="PSUM") as pb,
            tc.tile_pool(name="pc", bufs=1, space="PSUM") as pc,
        ):
            # DMA issue order tracks first use: w1 + xn0 (MM1 of chunk 0,
            # split so the k-outer chunk-0 MM1 starts early), xn1 (MM1 of
            # chunk 1), then w2 (MM2 of chunk 0).
            w1 = wp.tile([128, NT, CA_RED], _bf)
            xn0 = xp.tile([128, NT, NC], _bf, name="xn")
            nc.sync.dma_start(w1[:, :3, :], w1_d[:, :3, :])
            nc.sync.dma_start(xn0[:, :3, :], xn_d[:, :3, 0:NC])
            nc.sync.dma_start(w1[:, 3:9, :], w1_d[:, 3:9, :])
            nc.sync.dma_start(xn0[:, 3:8, :], xn_d[:, 3:8, 0:NC])
            nc.sync.dma_start(xn0[:, 8:13, :], xn_d[:, 8:13, 0:NC])
            nc.sync.dma_start(w1[:, 9:16, :], w1_d[:, 9:16, :])
            nc.sync.dma_start(xn0[:, 13:18, :], xn_d[:, 13:18, 0:NC])
            nc.sync.dma_start(w1[:, 16:, :], w1_d[:, 16:, :])
            nc.sync.dma_start(xn0[:, 18:, :], xn_d[:, 18:, 0:NC])
            b1 = wp.tile([128, 3], _f32)
            nc.sync.dma_start(b1[:], b1_d[:])
            xn1 = xp.tile([128, NT, NC], _bf, name="xn")
            for q0, q1 in [(0, 5), (5, 10), (10, 14), (14, 18), (18, NT)]:
                nc.sync.dma_start(xn1[:, q0:q1, :], xn_d[:, q0:q1, NC:2 * NC])
            w2 = wp.tile([128, 3, NT * 128], _bf)
            nc.sync.dma_start(w2[:], w2_d[:])
            b2 = wp.tile([128, NT], _f32)
            nc.sync.dma_start(b2[:], b2_d[:])
            w34 = wp.tile([128, NT1, G3], _bf)
            nc.sync.dma_start(w34[:], w34_d[:])
            b34 = wp.tile([G3, 1], _f32)
            nc.sync.dma_start(b34[:], b34_d[:])

            def mm1(xn, k_outer):
                # MM1: h1[320, NC] = relu(W1^T-chain), K = NT*128.
                # chunk 0 runs k-outer (3 live accumulators) so compute
                # overlaps the startup DMAs; later chunks run m-outer.
                h1 = hp.tile([128, 3, NC], _bf, name="h1")
                pss = [pa.tile([128, NC], _f32, name=f"ps1_{i}") for i in range(3)]
                order = (
                    [(kt, mi) for kt in range(NT) for mi in range(3)]
                    if k_outer else
                    [(kt, mi) for mi in range(3) for kt in range(NT)]
                )
                for kt, mi in order:
                    mo, msz = M1[mi]
                    nc.tensor.matmul(
                        pss[mi][:msz, :], w1[:, kt, mo:mo + msz], xn[:, kt, :],
                        start=(kt == 0), stop=(kt == NT - 1))
                for mi, (mo, msz) in enumerate(M1):
                    nc.scalar.activation(h1[:msz, mi, :], pss[mi][:msz, :],
                                         AF.Relu, bias=b1[:msz, mi:mi + 1])
                return h1

            # software pipeline: MM1 of chunk c+1 runs between MM1(c)'s
            # activations and MM2(c), so the PE never waits on the scalar
            # engine at the MM1->MM2 handoff.
            xns = [xn0, xn1]
            h1s = [mm1(xn0, True)]
            for c in range(NCHUNK):
                c0 = c * NC
                if c + 1 < NCHUNK:
                    h1s.append(mm1(xns[c + 1], c == 0))
                xn, h1 = xns[c], h1s[c]

                # MM2: g[2944, NC] = sigmoid(...), K = 320; m-tiles emitted in
                # M2_ORDER; prod[mt] = g[mt] * xn[mt] on DVE right after each
                # sigmoid so fuse pairs complete early.
                # MM2 + fuse: feat[j] = g[j]*xn[j] + g[11+j]*xn[11+j], all
                # partition-aligned under sigma. Fuse ops are emitted as soon
                # as each pair's sigmoids exist so the DVE tracks MM2.
                g = gp.tile([128, NT, NC], _bf)
                prodL = rp.tile([128, NC], _bf, name="prodL")
                prodM = rp.tile([64, NC], _bf, name="prodM")
                feat = fp.tile([128, NT1, NC], _bf)
                for mt in M2_ORDER:
                    mo2 = mt * 128
                    ps = pb.tile([128, NC], _f32, name="ps2")
                    for ki, ksz in K2:
                        nc.tensor.matmul(
                            ps[:, :], w2[:ksz, ki, mo2:mo2 + 128], h1[:ksz, ki, :],
                            start=(ki == 0), stop=(ki == 2))
                    nc.scalar.activation(g[:, mt, :], ps[:, :],
                                         AF.Sigmoid, bias=b2[:, mt:mt + 1])
                    if mt < 11:
                        nc.vector.tensor_mul(feat[:, mt, :], g[:, mt, :], xn[:, mt, :])
                    elif mt < 22:
                        j = mt - 11
                        tmp = tp.tile([128, NC], _bf)
                        nc.vector.tensor_mul(tmp[:], g[:, mt, :], xn[:, mt, :])
                        nc.vector.tensor_add(feat[:, j, :], feat[:, j, :], tmp[:])
                    else:
                        nc.vector.tensor_mul(prodL[:], g[:, 22, :], xn[:, 22, :])
                        # partition-shifted compute ops crash the device;
                        # move the mean half to base 0 with an SBUF->SBUF DMA
                        nc.sync.dma_start(prodM[:, :], prodL[64:128, :])

                # MM34 (ANN fc0 + GRU input proj folded): xg[96, NC].
                # K = 11 full feat tiles + two base-64 k-tiles of prodL (the
                # var/mean halves of features 1408..1471; PSUM sums them).
                # Last chunk runs in two halves so the final activation + DMA
                # tail overlaps the second half's matmuls.
                # last chunk: split 360/120 and give each piece its OWN
                # PSUM tile (pa's ps1_1 is idle then) — a shared tile's
                # coarse dependency made piece 2 wait on piece 1's act.
                halves = [(0, NC, None)] if c + 1 < NCHUNK else                     [(0, 360, None), (360, NC, "ps1_1")]
                xg = op.tile([G3, NC], _f32, name="xg")
                for h0, h1e, alt in halves:
                    if alt is None:
                        ps = pc.tile([128, NC], _f32, name="ps34")
                    else:
                        ps = pa.tile([128, NC], _f32, name=alt)
                    for kt in range(11):
                        nc.tensor.matmul(
                            ps[:G3, h0:h1e], w34[:, kt, :], feat[:, kt, h0:h1e],
                            start=(kt == 0), stop=False)
                    nc.tensor.matmul(ps[:G3, h0:h1e], w34[0:64, 11, :],
                                     prodL[0:64, h0:h1e], start=False, stop=False)
                    nc.tensor.matmul(ps[:G3, h0:h1e], w34[0:64, 11, :],
                                     prodM[:, h0:h1e], start=False, stop=True)
                    nc.scalar.activation(xg[:, h0:h1e], ps[:G3, h0:h1e],
                                         AF.Identity, bias=b34[:, 0:1])
                    nc.sync.dma_start(out_d[:, c0 + h0:c0 + h1e], xg[:, h0:h1e])
                if c + 2 < NCHUNK:
                    xn2 = xp.tile([128, NT, NC], _bf, name="xn")
                    nc.sync.dma_start(xn2[:], xn_d[:, :, c0 + 2 * NC:c0 + 3 * NC])
                    xns.append(xn2)

    nc.compile()
    return nc


_NC_CACHE = []
_LAST_RES = []


def make_dev_weights(ca_fc1_w, ca_fc1_b, ca_fc2_w, ca_fc2_b,
                     ann_w, ann_b, gru_w_ih, gru_b_ih):
    """Device-layout weight map (shared across cores)."""
    w1_dev = np.ascontiguousarray(
        ca_fc1_w[:, SIG].T.reshape(NT, 128, CA_RED).transpose(1, 0, 2))
    b1_dev = np.zeros((3, 128), np.float32)
    b1_dev.reshape(-1)[:CA_RED] = ca_fc1_b
    b1_dev = np.ascontiguousarray(b1_dev.T)                       # [128,3]

    w2_p = np.zeros((384, F2), np.float32)
    w2_p[:CA_RED] = ca_fc2_w[SIG, :].T
    w2_dev = np.ascontiguousarray(
        w2_p.reshape(3, 128, F2).transpose(1, 0, 2))              # [128,3,2944]
    b2_dev = np.ascontiguousarray(
        ca_fc2_b[SIG].reshape(NT, 128).T)                         # [128,NT]

    # ANN fc0 and the GRU input projection are both linear -> fold:
    # xg = feat @ (W_ih @ W_ann).T + (W_ih @ b_ann + b_ih)
    w34 = (gru_w_ih.astype(np.float64) @ ann_w.astype(np.float64)).astype(np.float32)
    b34 = (gru_w_ih.astype(np.float64) @ ann_b.astype(np.float64)
           + gru_b_ih.astype(np.float64)).astype(np.float32)
    w34_dev = np.zeros((128, NT1, G3), np.float32)
    w34_dev[:, :11, :] = w34[:, :1408].reshape(G3, 11, 128).transpose(2, 1, 0)
    w34_dev[0:64, 11, :] = w34[:, 1408:].T      # feats 1408..1471 (used twice)
    b34_dev = np.ascontiguousarray(b34.reshape(G3, 1))

    return {
        "w1": _rne_bf16(w1_dev).view(BF16),
        "b1": b1_dev,
        "w2": _rne_bf16(w2_dev).view(BF16),
        "b2": b2_dev,
        "w34": _rne_bf16(w34_dev).view(BF16),
        "b34": b34_dev,
    }


def make_xn_u16(flat, mean_var, std_var, mean_mean, std_mean):
    """[rows, F2] fp32 -> normalized bf16 bits [rows, F2] in sigma order."""
    rows = flat.shape[0]
    out = np.empty((rows, F2), np.uint16)
    inv_sv = (1.0 / std_var).astype(np.float32)
    inv_sm = (1.0 / std_mean).astype(np.float32)
    xv = _rne_bf16((flat[:, :F1] - mean_var) * inv_sv)
    xm = _rne_bf16((flat[:, F1:] - mean_mean) * inv_sm)
    out[:, 0:1408] = xv[:, :1408]
    out[:, 1408:2816] = xm[:, :1408]
    out[:, 2816:2880] = xv[:, 1408:]
    out[:, 2880:2944] = xm[:, 1408:]
    return out


def core_xn(xn_u16_rows):
    """[R, F2] u16 -> [128, NT, R] device layout."""
    return np.ascontiguousarray(
        xn_u16_rows.reshape(R, NT, 128).transpose(2, 1, 0))


def kernel(input, input_length, mean_var, std_var, mean_mean, std_mean,
           ca_fc1_w, ca_fc1_b, ca_fc2_w, ca_fc2_b, ann_w, ann_b,
           gru_w_ih, gru_w_hh, gru_b_ih, gru_b_hh, q_w, q_b,
           att_w1, att_b1, att_w2, att_b2, qreg2_w, qreg2_b):
    input = np.asarray(input, np.float32)

    xn_u = make_xn_u16(input.reshape(B * T, F2),
                       mean_var, std_var, mean_mean, std_mean)
    shared = make_dev_weights(ca_fc1_w, ca_fc1_b, ca_fc2_w, ca_fc2_b,
                              ann_w, ann_b, gru_w_ih, gru_b_ih)

    in_maps = []
    for c in range(NCORES):
        m = dict(shared)
        m["xn"] = core_xn(xn_u[c * R:(c + 1) * R]).view(BF16)
        in_maps.append(m)

    if not _NC_CACHE:
        _NC_CACHE.append(_build_bass())
    nc = _NC_CACHE[0]

    res = run_bass_kernel_spmd(nc, in_maps, core_ids=list(range(NCORES)))
    _LAST_RES.clear()
    _LAST_RES.append(res)
    xg = np.stack([r["xg"] for r in res.results])        # [NCORES,96,R]
    xg = np.transpose(xg, (0, 2, 1)).reshape(B, T, G3)   # [B,T,96]

    # --- host: GRU scan (tiny)
    w_hh_T = gru_w_hh.T.astype(np.float32)
    b_hh = gru_b_hh.astype(np.float32)
    h = np.zeros((B, H), np.float32)
    outs = np.empty((B, T, H), np.float32)
    for t in range(T):
        hg = h @ w_hh_T + b_hh
        xt = xg[:, t]
        r = 1.0 / (1.0 + np.exp(-(xt[:, :H] + hg[:, :H])))
        z = 1.0 / (1.0 + np.exp(-(xt[:, H:2 * H] + hg[:, H:2 * H])))
        n = np.tanh(xt[:, 2 * H:] + r * hg[:, 2 * H:])
        h = (1.0 - z) * n + z * h
        outs[:, t] = h

    # --- temporal attention convs
    sw = np.lib.stride_tricks.sliding_window_view
    win1 = sw(outs, K, axis=1)                           # [B,T-14,H,K]
    a = np.einsum("btck,ck->bt", win1, att_w1[0], optimize=True) + att_b1[0]
    a = np.maximum(a, 0.0)
    win2 = sw(a, K, axis=1)                              # [B,N_CROP,K]
    a = win2 @ att_w2[0, 0] + att_b2[0]
    att = np.tanh(a)                                     # [B,N_CROP]

    q_fet = att[..., None] * outs[:, K - 1:T - (K - 1), :]   # [B,N_CROP,H]

    P = _pool_matrix(N_CROP, PYR)                        # [126,N_CROP]
    row0 = q_fet.mean(1, keepdims=True)
    num = np.einsum("mn,bnh->bmh", P, q_fet, optimize=True)
    den = (att @ P.T)[..., None]
    q_pool = np.concatenate([row0, num / den], 1)        # [B,127,H]

    q_each = (q_pool @ q_w.T + q_b)[..., 0]              # [B,127]
    score = q_each @ qreg2_w.T + qreg2_b
    return score.astype(np.float32)


# revision 7
# speedup vs baseline: 1.0185x; 1.0075x over previous
import sys
import numpy as np

sys.path.insert(0, "/opt/trn_rl_repo")

from concourse import bacc, bass, tile
from concourse.bass_utils import run_bass_kernel_spmd

mybir = bass.mybir
import ml_dtypes

BF16 = ml_dtypes.bfloat16

B, T, F2, F1 = 64, 300, 2944, 1472
RED, H, K, LN = 256, 32, 15, 7
CA_RED = 320
N_CROP = T - 2 * (K - 1)
PYR = [2 ** j for j in range(1, LN)]

NCORES = 8
V = B // NCORES          # videos per core
R = V * T                # rows per core (2400)
NT = 23                  # xn k-tiles: exact var|mean, 23*128 == 2944
NT1 = 12                 # feat k-tiles (tile 11 only has 64 live rows)
NC = 480                 # column chunk (free dim per matmul)
NCHUNK = R // NC
G3 = 3 * H               # 96

# Feature order on device is permuted so that var/mean fuse pairs are
# partition-aligned: sigma = [var 0..1407 | mean 0..1407 | var 1408..1471 |
# mean 1408..1471]. Pair j pairs tile j with tile 11+j; tile 22 holds the
# last 64 features' var half (partitions 0..63) and mean half (64..127),
# which MM34 consumes as two base-64 k-tiles (the PSUM accumulate sums them).
SIG = np.concatenate([np.arange(0, 1408), 1472 + np.arange(0, 1408),
                      np.arange(1408, 1472), np.arange(2880, 2944)])

# MM2 m-tile emission order: tile 22 first (its prodL feeds MM34 via an
# SBUF->SBUF DMA with ~2us latency), then fuse pairs (j, 11+j).
M2_ORDER = [22] + [v for _j in range(11) for v in (_j, 11 + _j)]
assert sorted(M2_ORDER) == list(range(NT))

_bf = mybir.dt.bfloat16
_f32 = mybir.dt.float32


def _pool_matrix(N, sizes):
    mats = []
    for m in sizes:
        P = np.zeros((m, N), np.float32)
        for i in range(m):
            s = (i * N) // m
            e = -((-(i + 1) * N) // m)
            P[i, s:e] = 1.0 / (e - s)
        mats.append(P)
    return np.concatenate(mats, 0)


def _rne_bf16(x):
    """fp32 ndarray -> uint16 bf16 bits, round-to-nearest-even."""
    u = np.ascontiguousarray(x, np.float32).view(np.uint32)
    return ((u + 0x7FFF + ((u >> 16) & 1)) >> 16).astype(np.uint16)


def _build_bass():
    nc = bacc.Bacc(None, target_bir_lowering=False)

    xn_d = nc.dram_tensor("xn", [128, NT, R], _bf, kind="ExternalInput")
    w1_d = nc.dram_tensor("w1", [128, NT, CA_RED], _bf, kind="ExternalInput")
    b1_d = nc.dram_tensor("b1", [128, 3], _f32, kind="ExternalInput")
    w2_d = nc.dram_tensor("w2", [128, 3, NT * 128], _bf, kind="ExternalInput")
    b2_d = nc.dram_tensor("b2", [128, NT], _f32, kind="ExternalInput")
    w34_d = nc.dram_tensor("w34", [128, NT1, G3], _bf, kind="ExternalInput")
    b34_d = nc.dram_tensor("b34", [G3, 1], _f32, kind="ExternalInput")
    out_d = nc.dram_tensor("xg", [G3, R], _f32, kind="ExternalOutput")

    AF = mybir.ActivationFunctionType
    M1 = [(0, 128), (128, 128), (256, 64)]   # MM1 m-tiles over CA_RED=320
    K2 = [(0, 128), (1, 128), (2, 64)]       # MM2 k-tiles over 320

    with tile.TileContext(nc) as tc:
        with (
            tc.tile_pool(name="wp", bufs=1) as wp,
            tc.tile_pool(name="xp", bufs=2) as xp,
            tc.tile_pool(name="hp", bufs=2) as hp,
            tc.tile_pool(name="gp", bufs=2) as gp,
            tc.tile_pool(name="rp", bufs=2) as rp,
            tc.tile_pool(name="fp", bufs=2) as fp,
            tc.tile_pool(name="tp", bufs=3) as tp,
            tc.tile_pool(name="op", bufs=2) as op,
            tc.tile_pool(name="pa", bufs=1, space="PSUM") as pa,
            tc.tile_pool(name="pb", bufs=4, space="PSUM") as pb,
            tc.tile_pool(name="pc", bufs=1, space="PSUM") as pc,
        ):
            # DMA issue order tracks first use: w1 + xn0 (MM1 of chunk 0,
            # split so the k-outer chunk-0 MM1 starts early), xn1 (MM1 of
            # chunk 1), then w2 (MM2 of chunk 0).
            w1 = wp.tile([128, NT, CA_RED], _bf)
            xn0 = xp.tile([128, NT, NC], _bf, name="xn")
            nc.sync.dma_start(w1[:, :2, :], w1_d[:, :2, :])
            nc.sync.dma_start(xn0[:, :2, :], xn_d[:, :2, 0:NC])
            nc.sync.dma_start(w1[:, 2:9, :], w1_d[:, 2:9, :])
            nc.sync.dma_start(xn0[:, 2:3, :], xn_d[:, 2:3, 0:NC])
            nc.sync.dma_start(xn0[:, 3:8, :], xn_d[:, 3:8, 0:NC])
            nc.sync.dma_start(xn0[:, 8:13, :], xn_d[:, 8:13, 0:NC])
            nc.sync.dma_start(w1[:, 9:16, :], w1_d[:, 9:16, :])
            nc.sync.dma_start(xn0[:, 13:18, :], xn_d[:, 13:18, 0:NC])
            nc.sync.dma_start(w1[:, 16:, :], w1_d[:, 16:, :])
            nc.sync.dma_start(xn0[:, 18:, :], xn_d[:, 18:, 0:NC])
            b1 = wp.tile([128, 3], _f32)
            nc.sync.dma_start(b1[:], b1_d[:])
            xn1 = xp.tile([128, NT, NC], _bf, name="xn")
            for q0, q1 in [(0, 5), (5, 10), (10, 14), (14, 18), (18, NT)]:
                nc.sync.dma_start(xn1[:, q0:q1, :], xn_d[:, q0:q1, NC:2 * NC])
            w2 = wp.tile([128, 3, NT * 128], _bf)
            nc.sync.dma_start(w2[:], w2_d[:])
            b2 = wp.tile([128, NT], _f32)
            nc.sync.dma_start(b2[:], b2_d[:])
            w34 = wp.tile([128, NT1, G3], _bf)
            nc.sync.dma_start(w34[:], w34_d[:])
            b34 = wp.tile([G3, 1], _f32)
            nc.sync.dma_start(b34[:], b34_d[:])

            def mm1(xn, k_outer):
                # MM1: h1[320, NC] = relu(W1^T-chain), K = NT*128.
                # chunk 0 runs k-outer (3 live accumulators) so compute
                # overlaps the startup DMAs; later chunks run m-outer.
                h1 = hp.tile([128, 3, NC], _bf, name="h1")
                pss = [pa.tile([128, NC], _f32, name=f"ps1_{i}") for i in range(3)]
                order = (
                    [(kt, mi) for kt in range(NT) for mi in range(3)]
                    if k_outer else
                    [(kt, mi) for mi in range(3) for kt in range(NT)]
                )
                for kt, mi in order:
                    mo, msz = M1[mi]
                    nc.tensor.matmul(
                        pss[mi][:msz, :], w1[:, kt, mo:mo + msz], xn[:, kt, :],
                        start=(kt == 0), stop=(kt == NT - 1))
                for mi, (mo, msz) in enumerate(M1):
                    nc.scalar.activation(h1[:msz, mi, :], pss[mi][:msz, :],
                                         AF.Relu, bias=b1[:msz, mi:mi + 1])
                return h1

            # software pipeline: MM1 of chunk c+1 runs between MM1(c)'s
            # activations and MM2(c), so the PE never waits on the scalar
            # engine at the MM1->MM2 handoff.
            xns = [xn0, xn1]
            h1s = [mm1(xn0, True)]
            for c in range(NCHUNK):
                c0 = c * NC
                if c + 1 < NCHUNK:
                    h1s.append(mm1(xns[c + 1], c == 0))
                xn, h1 = xns[c], h1s[c]

                # MM2: g[2944, NC] = sigmoid(...), K = 320; m-tiles emitted in
                # M2_ORDER; prod[mt] = g[mt] * xn[mt] on DVE right after each
                # sigmoid so fuse pairs complete early.
                # MM2 + fuse: feat[j] = g[j]*xn[j] + g[11+j]*xn[11+j], all
                # partition-aligned under sigma. Fuse ops are emitted as soon
                # as each pair's sigmoids exist so the DVE tracks MM2.
                g = gp.tile([128, NT, NC], _bf)
                prodL = rp.tile([128, NC], _bf, name="prodL")
                prodM = rp.tile([64, NC], _bf, name="prodM")
                feat = fp.tile([128, NT1, NC], _bf)
                for mt in M2_ORDER:
                    mo2 = mt * 128
                    ps = pb.tile([128, NC], _f32, name="ps2")
                    for ki, ksz in K2:
                        nc.tensor.matmul(
                            ps[:, :], w2[:ksz, ki, mo2:mo2 + 128], h1[:ksz, ki, :],
                            start=(ki == 0), stop=(ki == 2))
                    nc.scalar.activation(g[:, mt, :], ps[:, :],
                                         AF.Sigmoid, bias=b2[:, mt:mt + 1])
                    if mt < 11:
                        nc.vector.tensor_mul(feat[:, mt, :], g[:, mt, :], xn[:, mt, :])
                    elif mt < 22:
                        j = mt - 11
                        tmp = tp.tile([128, NC], _bf)
                        nc.vector.tensor_mul(tmp[:], g[:, mt, :], xn[:, mt, :])
                        nc.vector.tensor_add(feat[:, j, :], feat[:, j, :], tmp[:])
                    else:
                        nc.vector.tensor_mul(prodL[:], g[:, 22, :], xn[:, 22, :])
                        # partition-shifted compute ops crash the device;
                        # move the mean half to base 0 with an SBUF->SBUF DMA
                        nc.sync.dma_start(prodM[:, :], prodL[64:128, :])

                # MM34 (ANN fc0 + GRU input proj folded): xg[96, NC].
                # K = 11 full feat tiles + two base-64 k-tiles of prodL (the
                # var/mean halves of features 1408..1471; PSUM sums them).
                # Last chunk runs in two halves so the final activation + DMA
                # tail overlaps the second half's matmuls.
                # last chunk: split 360/120 and give each piece its OWN
                # PSUM tile (pa's ps1_1 is idle then) — a shared tile's
                # coarse dependency made piece 2 wait on piece 1's act.
                halves = [(0, NC, None)] if c + 1 < NCHUNK else                     [(0, 360, None), (360, NC, "ps1_1")]
                xg = op.tile([G3, NC], _f32, name="xg")
                for h0, h1e, alt in halves:
                    if alt is None:
                        ps = pc.tile([128, NC], _f32, name="ps34")
                    else:
                        ps = pa.tile([128, NC], _f32, name=alt)
                    for kt in range(11):
                        nc.tensor.matmul(
                            ps[:G3, h0:h1e], w34[:, kt, :], feat[:, kt, h0:h1e],
                            start=(kt == 0), stop=False)
                    nc.tensor.matmul(ps[:G3, h0:h1e], w34[0:64, 11, :],
                                     prodL[0:64, h0:h1e], start=False, stop=False)
                    nc.tensor.matmul(ps[:G3, h0:h1e], w34[0:64, 11, :],
                                     prodM[:, h0:h1e], start=False, stop=True)
                    nc.scalar.activation(xg[:, h0:h1e], ps[:G3, h0:h1e],
                                         AF.Identity, bias=b34[:, 0:1])
                    nc.sync.dma_start(out_d[:, c0 + h0:c0 + h1e], xg[:, h0:h1e])
                if c + 2 < NCHUNK:
                    xn2 = xp.tile([128, NT, NC], _bf, name="xn")
                    nc.sync.dma_start(xn2[:], xn_d[:, :, c0 + 2 * NC:c0 + 3 * NC])
                    xns.append(xn2)

    nc.compile()
    return nc


_NC_CACHE = []
_LAST_RES = []


def make_dev_weights(ca_fc1_w, ca_fc1_b, ca_fc2_w, ca_fc2_b,
                     ann_w, ann_b, gru_w_ih, gru_b_ih):
    """Device-layout weight map (shared across cores)."""
    w1_dev = np.ascontiguousarray(
        ca_fc1_w[:, SIG].T.reshape(NT, 128, CA_RED).transpose(1, 0, 2))
    b1_dev = np.zeros((3, 128), np.float32)
    b1_dev.reshape(-1)[:CA_RED] = ca_fc1_b
    b1_dev = np.ascontiguousarray(b1_dev.T)                       # [128,3]

    w2_p = np.zeros((384, F2), np.float32)
    w2_p[:CA_RED] = ca_fc2_w[SIG, :].T
    w2_dev = np.ascontiguousarray(
        w2_p.reshape(3, 128, F2).transpose(1, 0, 2))              # [128,3,2944]
    b2_dev = np.ascontiguousarray(
        ca_fc2_b[SIG].reshape(NT, 128).T)                         # [128,NT]

    # ANN fc0 and the GRU input projection are both linear -> fold:
    # xg = feat @ (W_ih @ W_ann).T + (W_ih @ b_ann + b_ih)
    w34 = (gru_w_ih.astype(np.float64) @ ann_w.astype(np.float64)).astype(np.float32)
    b34 = (gru_w_ih.astype(np.float64) @ ann_b.astype(np.float64)
           + gru_b_ih.astype(np.float64)).astype(np.float32)
    w34_dev = np.zeros((128, NT1, G3), np.float32)
    w34_dev[:, :11, :] = w34[:, :1408].reshape(G3, 11, 128).transpose(2, 1, 0)
    w34_dev[0:64, 11, :] = w34[:, 1408:].T      # feats 1408..1471 (used twice)
    b34_dev = np.ascontiguousarray(b34.reshape(G3, 1))

    return {
        "w1": _rne_bf16(w1_dev).view(BF16),
        "b1": b1_dev,
        "w2": _rne_bf16(w2_dev).view(BF16),
        "b2": b2_dev,
        "w34": _rne_bf16(w34_dev).view(BF16),
        "b34": b34_dev,
    }


def make_xn_u16(flat, mean_var, std_var, mean_mean, std_mean):
    """[rows, F2] fp32 -> normalized bf16 bits [rows, F2] in sigma order."""
    rows = flat.shape[0]
    out = np.empty((rows, F2), np.uint16)
    inv_sv = (1.0 / std_var).astype(np.float32)
    inv_sm = (1.0 / std_mean).astype(np.float32)
    xv = _rne_bf16((flat[:, :F1] - mean_var) * inv_sv)
    xm = _rne_bf16((flat[:, F1:] - mean_mean) * inv_sm)
    out[:, 0:1408] = xv[:, :1408]
    out[:, 1408:2816] = xm[:, :1408]
    out[:, 2816:2880] = xv[:, 1408:]
    out[:, 2880:2944] = xm[:, 1408:]
    return out


def core_xn(xn_u16_rows):
    """[R, F2] u16 -> [128, NT, R] device layout."""
    return np.ascontiguousarray(
        xn_u16_rows.reshape(R, NT, 128).transpose(2, 1, 0))


def kernel(input, input_length, mean_var, std_var, mean_mean, std_mean,
           ca_fc1_w, ca_fc1_b, ca_fc2_w, ca_fc2_b, ann_w, ann_b,
           gru_w_ih, gru_w_hh, gru_b_ih, gru_b_hh, q_w, q_b,
           att_w1, att_b1, att_w2, att_b2, qreg2_w, qreg2_b):
    input = np.asarray(input, np.float32)

    xn_u = make_xn_u16(input.reshape(B * T, F2),
                       mean_var, std_var, mean_mean, std_mean)
    shared = make_dev_weights(ca_fc1_w, ca_fc1_b, ca_fc2_w, ca_fc2_b,
                              ann_w, ann_b, gru_w_ih, gru_b_ih)

    in_maps = []
    for c in range(NCORES):
        m = dict(shared)
        m["xn"] = core_xn(xn_u[c * R:(c + 1) * R]).view(BF16)
        in_maps.append(m)

    if not _NC_CACHE:
        _NC_CACHE.append(_build_bass())
    nc = _NC_CACHE[0]

    res = run_bass_kernel_spmd(nc, in_maps, core_ids=list(range(NCORES)))
    _LAST_RES.clear()
    _LAST_RES.append(res)
    xg = np.stack([r["xg"] for r in res.results])        # [NCORES,96,R]
    xg = np.transpose(xg, (0, 2, 1)).reshape(B, T, G3)   # [B,T,96]

    # --- host: GRU scan (tiny)
    w_hh_T = gru_w_hh.T.astype(np.float32)
    b_hh = gru_b_hh.astype(np.float32)
    h = np.zeros((B, H), np.float32)
    outs = np.empty((B, T, H), np.float32)
    for t in range(T):
        hg = h @ w_hh_T + b_hh
        xt = xg[:, t]
        r = 1.0 / (1.0 + np.exp(-(xt[:, :H] + hg[:, :H])))
        z = 1.0 / (1.0 + np.exp(-(xt[:, H:2 * H] + hg[:, H:2 * H])))
        n = np.tanh(xt[:, 2 * H:] + r * hg[:, 2 * H:])
        h = (1.0 - z) * n + z * h
        outs[:, t] = h

    # --- temporal attention convs
    sw = np.lib.stride_tricks.sliding_window_view
    win1 = sw(outs, K, axis=1)                           # [B,T-14,H,K]
    a = np.einsum("btck,ck->bt", win1, att_w1[0], optimize=True) + att_b1[0]
    a = np.maximum(a, 0.0)
    win2 = sw(a, K, axis=1)                              # [B,N_CROP,K]
    a = win2 @ att_w2[0, 0] + att_b2[0]
    att = np.tanh(a)                                     # [B,N_CROP]

    q_fet = att[..., None] * outs[:, K - 1:T - (K - 1), :]   # [B,N_CROP,H]

    P = _pool_matrix(N_CROP, PYR)                        # [126,N_CROP]
    row0 = q_fet.mean(1, keepdims=True)
    num = np.einsum("mn,bnh->bmh", P, q_fet, optimize=True)
    den = (att @ P.T)[..., None]
    q_pool = np.concatenate([row0, num / den], 1)        # [B,127,H]

    q_each = (q_pool @ q_w.T + q_b)[..., 0]              # [B,127]
    score = q_each @ qreg2_w.T + qreg2_b
    return score.astype(np.float32)
